# revision 50
# baseline (speedup 1.0000x reference)
"""Trainium2 Bass kernel for nn_LookAtMappingNetwork (gnn_message_passing).

Strategy
--------
The module's output only reads the final node features at rows R = {i*250 :
i in 0..63} (``ws = x[::250]``).  Working backwards through the two message
-passing processors, only a small data-dependent subset of edges/nodes can
influence those rows, for ANY edge_index:

    E1 = edges with dst in R          (<= 58 per core)  -> proc-1 edge MLP
    S  = R  U  src[E1]                (<= 65 per core)  -> rows where x1 needed
    E0 = edges with dst in S          (<= 375 per core) -> proc-0 edge MLP

Segment-mean counts stay exact because E0/E1 contain ALL edges landing on
S/R.  Everything else the reference computes is dead code.  Each of the 8
cores handles 8 output rows (its R_c) fully independently.

Performance layout
------------------
* All FC weights are transposed, pre-scaled by (lr/sqrt(fan_in))*sqrt(2)
  and packed host-side into ONE bf16 tensor of 128-row K-tiles (pair-
  interleaved in DRAM so each DMA line is a contiguous 2 KiB run).  Since
  leaky_relu commutes with positive scaling, each layer's activation
  collapses to copy+max on Scalar/DVE with zero extra scalar work.
* Weight DMA is chained on the sync ring in first-use order via WAW
  overlaps on late-consumed tiles (GA -> GB1 -> GB2 -> GC), so each layer's
  weights land just-in-time while compute streams.
* Metadata rides in just TWO tensors (each dma_start costs ~1.3us of
  serial ring latency): mzc [128 x F] carries z + per-partition values +
  bf16 gather matrices bit-packed into f32 columns; mgr [3 x F] carries
  feature-major la geometry + all index VALUE rows.  Index rows are
  broadcast on-chip with fp16 one-row PE matmuls (exact for these small
  ints, 4x faster than f32) and compared against iota on the DVE.
* Mean-aggregation gather matrices G0R/G1R ship host-side with 1/count
  pre-folded (bf16): aggregation becomes plain feature-major matmuls --
  no token-major agg psum, no DVE rescale, no PE transposes.
* The back half (n00 / e10 / n10) is FEATURE-major: matmul cost is the
  moving-row count, so N drops from 512 to 66/64/8 per matmul (30-60ns
  chained), the fm lrelu chunks are directly the lhsT of the next
  token-major layer (zero transposes), and per-partition biases ride the
  scalar-engine activation staging.  Only layers feeding an aggregation
  or gather (msg, n01, e11) stay token-major, since those contractions
  need tokens on the partition axis.
* PSUM accumulation groups are fc-sequential (one open group per 2 KiB
  zero-region/bank); leaky-relu runs as copy+max staged across
  Scalar/DVE in 256-col chunks with the next layer's consumers emitted
  inline.
* CAP_S=66 / CAP_E1=64 (actual maxima 65/58) halve the mid-kernel token
  dimensions vs the padded-to-128 v1.
* w1e0 (12 tiles) ships as fp8_e4m3 (scaled 2^6; the e10 lrelu rescales by
  2^-6 for free) on the scalar ring.
* Output is written un-replicated [8, 512]; the x14 ws broadcast happens
  on the host.
"""

import math

import ml_dtypes
import numpy as np

import concourse.bacc as bacc
import concourse.bass as bass
import concourse.mybir as mybir
import concourse.tile as tile
from concourse.bass_utils import run_bass_kernel_spmd
from concourse.masks import make_identity

f32 = mybir.dt.float32
fr = mybir.dt.bfloat16
f16 = mybir.dt.float16
i32 = mybir.dt.int32
AF = mybir.ActivationFunctionType
OP = mybir.AluOpType

NV = 250
B = 64
D = 512
LR = 0.01
SQ2 = math.sqrt(2.0)
N_CORES = 8
R_PER = B // N_CORES  # output rows per core

CAP_E0 = 384
CAP_S = 66
CAP_E1 = 64
NT0 = CAP_E0 // 128

G_E00 = LR / math.sqrt(1034.0)
G_E01 = LR / math.sqrt(512.0)
G_N00 = LR / math.sqrt(1030.0)
G_N01 = LR / math.sqrt(512.0)
G_E10 = LR / math.sqrt(1536.0)
G_E11 = LR / math.sqrt(512.0)
G_N10 = LR / math.sqrt(1024.0)
G_N11 = LR / math.sqrt(512.0)

# ---- packed weight tile indices (each tile = [128, 512] bf16) ----
T_ZSRC = 0     # 4 tiles: w0e0^T rows 0:512    (z of src)
T_ZDST = 4     # 4 tiles: w0e0^T rows 515:1027 (z of dst)
# pads 8, 9 (G0b overlap)
T_LARAW = 10   # 0:3 laA-rel | 32:35 laB+rel | 64:65 wd | 96:97 b_e00
T_BROWS_E = 11  # bias rows for e01@0 / n01@32 / e10@64
T_W0E1 = 12    # 4
T_BROWS_L = 16  # bias rows for e11@0 / n10@32 / n11@64 (late-consumed)
T_N00C = 17    # 0:3 la | 32:35 la_dst-mean | 64:65 b_n00 (late-consumed)
T_W0N0Z = 18   # 4: w0n0^T rows 0:512
T_W0N0A = 22   # 4: w0n0^T rows 518:1030 (ef-mean block)
T_W0N1 = 26    # 4
T_W1E1 = 30    # 4
T_W1N0 = 34    # 8
T_W1N1 = 42    # 4
NT = 46
# chain groups: G0a=[0:10) -> G0b=[8:18) -> G2=[16:34) -> G3=[32:46)
#   G2 rewrites 16,17 (BROWS_L, N00C: consumed at n00/e11/n10/n11)
#   G3 rewrites 32,33 (W1E1 tiles 2,3: consumed at e11, ~when G3 lands)
# The fp8 pack is issued on the sync ring between G2 and G3: its config
# waits behind G2's chain wait, so it streams in the post-G2 window
# without stealing bandwidth from the critical bf16 stream.

T8_W1E0 = 0    # 12 tiles (fp8 pack)
NT8 = 12
F8S = 64.0

# brow key -> (tile, partition base): matmul bases must be 0/32/64
BROW_SLOT = {"e01": (T_BROWS_E, 0), "n01": (T_BROWS_E, 32),
             "e10": (T_BROWS_E, 64), "e11": (T_BROWS_L, 0),
             "n10": (T_BROWS_L, 32), "n11": (T_BROWS_L, 64)}

# ---- mzc [128, MZC_F] f32: z + per-partition values + packed bf16 ----
# rows 0:64 cols 0:512 = z
CZ_LDST = 512   # 9 cols: look_ats[e0 dst] token-major, 3 per e-tile
CZ_LAS = 521    # 3 cols: look_ats[S] token-major, rows 0:CAP_S
CZ_G1R = 524    # 4 f32 = 8 bf16: G1R [CAP_E1 rows, R_PER] (rin1 folded)
CZ_G0R = 528    # 99 f32 = 198 bf16: G0R [128 rows, 66*3] (rin0 folded)
CZ_BE10 = 627   # 4 cols: p1_eb0 * LR*sqrt2, feature-major [128, 4]
CZ_BN10 = 631   # 4 cols: p1_nb0 * LR*sqrt2, feature-major [128, 4]
CZ_BN11 = 635   # 4 cols: p1_nb1 * LR*sqrt2, feature-major [128, 4]
MZC_F = 639

# ---- mgr [3, MGR_F] f32: feature-major geometry + index value rows ----
GEO_S = 0       # [0:3, 0:384] la[e0 src]^T
GEO_D = 384     # [0:3, 384:768] la[e0 dst]^T
MR0 = 768       # row 0 only: packed index-value rows (see MR_* below)
MGR_F = 1800  # = 8 * 225; shipped as [24, 225] for DMA parallelism
# offsets within the fp16-cast copy mgr_h [1, 1026]
MR_E0GS = 0     # 384: e0 src % B
MR_SSEL = 384   # 66: S % B
MR_E0GD = 450   # 384: e0 dst % B
MR_E1POS = 834  # 64: e1 -> position in E0
MR_E1SRC = 898  # 64
MR_E1DST = 962  # 64
MR_N = 1026

DEBUG_DUMPS = False  # set True to add dbg_* DRAM dumps of intermediates


def _build_program():
    nc = bacc.Bacc("TRN2", target_bir_lowering=False, debug=False,
                   enable_asserts=False, num_devices=N_CORES)

    wpack_d = nc.dram_tensor("wpack", [NT * 128, 512], fr, kind="ExternalInput")
    wpack8_d = nc.dram_tensor("wpack8", [NT8 * 128, 512], mybir.dt.float8e4,
                              kind="ExternalInput")
    mzc_d = nc.dram_tensor("mzc", [128, MZC_F], f32, kind="ExternalInput")
    mgr_d = nc.dram_tensor("mgr", [24, MGR_F // 8], f32,
                           kind="ExternalInput")
    out_d = nc.dram_tensor("out", [R_PER, D], f32, kind="ExternalOutput")

    with tile.TileContext(nc) as tc, \
            tc.tile_pool(name="w", bufs=1) as wp, \
            tc.tile_pool(name="tmp", bufs=8) as tp, \
            tc.tile_pool(name="psb", bufs=2, space="PSUM") as psb, \
            tc.tile_pool(name="pss", bufs=6, space="PSUM") as pss:

        # ---------------- input DMAs ---------------------------------
        # scalar ring: meta first (small, unblocks the front-end), then fp8.
        mgr = wp.tile([3, MGR_F], f32, name="mgr")
        nc.scalar.dma_start(
            mgr[:, :].rearrange("p (k c) -> p k c", k=8),
            mgr_d[:, :].rearrange("(p k) c -> p k c", p=3))
        mzc = wp.tile([128, MZC_F], f32, name="mzc")
        nc.scalar.dma_start(mzc[:], mzc_d[:, :])

        wbig = wp.tile([128, NT, 512], fr, name="wbig")
        wbig8 = wp.tile([128, NT8, 512], mybir.dt.float8e4, name="wbig8")

        def wload(eng, a, b_):
            eng.dma_start(
                wbig[:, a:b_, :].rearrange("p (q j) d -> p q j d", j=2),
                wpack_d[128 * a:128 * b_, :].rearrange(
                    "(q p j) d -> p q j d", p=128, j=2))

        # Chained on the sync ring in first-use order via WAW overlaps.
        wload(nc.sync, 0, 10)          # z (+pads)
        wload(nc.sync, 8, 18)          # laraw, brows_e, w0e1, brows_l, n00c
        wload(nc.sync, 16, 34)         # w0n0, w0n1, w1e1
        wload(nc.sync, 32, 46)         # w1n0, w1n1
        nc.sync.dma_start(
            wbig8[:, :, :].rearrange("p (q j) d -> p q j d", j=4),
            wpack8_d[:, :].rearrange("(q p j) d -> p q j d", p=128, j=4))


        def W8(i):
            return wbig8[:, i, :]

        def W(i):
            return wbig[:, i, :]

        # bf16 gather matrices bit-packed in mzc's f32 columns
        G0R = mzc[:, CZ_G0R:CZ_G0R + 99].bitcast(fr)     # [128, 198]
        G1R = mzc[0:CAP_E1, CZ_G1R:CZ_G1R + 4].bitcast(fr)  # [64, 8]

        # ---------------- constants ----------------
        ident_f = wp.tile([128, 128], f32, name="ident_f")
        make_identity(nc, ident_f[:])
        ident = wp.tile([128, 128], fr, name="ident")
        nc.vector.tensor_copy(ident[:], ident_f[:])
        idents = {fr: ident, f32: ident_f}
        ones_f32 = wp.tile([128, 1], f32, name="ones_f32")
        nc.gpsimd.memset(ones_f32[:], 1.0)
        ones_h = wp.tile([1, 128], f16, name="ones_h")
        nc.gpsimd.memset(ones_h[:], 1.0)
        iota_part = []
        for t in range(NT0):
            it = wp.tile([128, 1], f32, name=f"iota_part{t}")
            nc.gpsimd.iota(it[:], pattern=[[1, 1]], base=128 * t,
                           channel_multiplier=1,
                           allow_small_or_imprecise_dtypes=True)
            iota_part.append(it)
        # ones rows at partition bases 0/32/64 (for bias-row matmuls)
        ones_rows = wp.tile([65, 128], fr, name="ones_rows")
        nc.vector.tensor_copy(ones_rows[:], ones_f32[:65, :1].to_broadcast([65, 128]))

        _uid = [0]

        def uid():
            _uid[0] += 1
            return _uid[0]

        def sb(shape, name):
            return wp.tile(shape, fr, name=name)

        _cp = [0]

        def ps_copy(dst_ap, src_ap):
            """PSUM->SBUF copy, alternating Vector/Scalar engines."""
            _cp[0] += 1
            if _cp[0] % 2 == 0:
                nc.vector.tensor_copy(dst_ap, src_ap)
            else:
                nc.scalar.copy(dst_ap, src_ap)

        def copyT(src_ap, p, f, dst_ap):
            """PE transpose src [p, f] -> existing sbuf dst_ap [f, p]."""
            sdt = src_ap.dtype
            ps = pss.tile([f, p], sdt, name=f"psT{uid()}", tag="pssm")
            nc.tensor.transpose(ps[:], src_ap, idents[sdt][:p, :p])
            ps_copy(dst_ap, ps[:])

        def peT(src_ap, p, f, name):
            dst = sb([f, p], name)
            copyT(src_ap, p, f, dst[:])
            return dst

        def brow_mm(ps_t, key, p):
            tidx, pbase = BROW_SLOT[key]
            nc.tensor.matmul(ps_t[:], ones_rows[pbase:pbase + 1, :p],
                             wbig[pbase:pbase + 1, tidx, :],
                             start=True, stop=False)

        def lrelu(ps_ap, out_ap, s_copy=False):
            """out = leaky_relu(psum, 0.2) -- gain pre-folded into weights.
            (The DVE cannot read two PSUM operands, so stage through SBUF.)"""
            p, n = ps_ap.shape
            t = tp.tile([p, n], f32, name=f"lr{uid()}", tag=f"lr{p}_{n}")
            if s_copy:
                nc.scalar.copy(t[:], ps_ap)
            else:
                nc.vector.tensor_copy(t[:], ps_ap)
            nc.vector.scalar_tensor_tensor(out_ap, t[:], 0.2, ps_ap,
                                           op0=OP.mult, op1=OP.max)

        def lrelu_chunk(ps_t, out_t, p, consume, scale=None):
            """Chunked lrelu over 4 x 128 output columns; consume(c, out_ap)
            emits the chunk's consumers right away so the PE restarts while
            later chunks are still on the DVE."""
            for c in range(4):
                cs = slice(128 * c, 128 * (c + 1))
                t = tp.tile([p, 128], f32, name=f"lrc{uid()}", tag=f"lrc{p}")
                if scale is None:
                    if c == 0:
                        nc.vector.tensor_copy(t[:], ps_t[:, cs])
                    else:
                        nc.scalar.copy(t[:], ps_t[:, cs])
                    nc.vector.scalar_tensor_tensor(out_t[:, cs], t[:], 0.2,
                                                   ps_t[:, cs],
                                                   op0=OP.mult, op1=OP.max)
                else:
                    if c == 0:
                        nc.vector.tensor_scalar_mul(t[:], ps_t[:, cs], scale)
                    else:
                        nc.scalar.activation(t[:], ps_t[:, cs], AF.Identity,
                                             bias=0.0, scale=scale)
                    nc.vector.scalar_tensor_tensor(out_t[:, cs], t[:], 0.2,
                                                   t[:],
                                                   op0=OP.mult, op1=OP.max)
                consume(c, out_t[:, cs])

        def iseq(out_ap, in_ap, iota_t):
            nc.vector.tensor_scalar(out_ap, in_ap, iota_t, None, OP.is_equal)

        mz = mzc[0:64, 0:512]

        # geometry first on the DVE (mgr lands well before mzc)
        rel = tp.tile([3, CAP_E0], f32, name="rel", tag="rel")
        nc.vector.tensor_tensor(rel[:], mgr[0:3, GEO_D:GEO_D + CAP_E0],
                                mgr[0:3, GEO_S:GEO_S + CAP_E0],
                                op=OP.subtract)
        sqr = sb([3, CAP_E0], "sqr")
        nc.vector.tensor_tensor(sqr[:], rel[:], rel[:], op=OP.mult)

        # dummy sqrt: forces the sqrt_and_friends act table (which also
        # serves Copy/Identity) to load ONCE in the preamble instead of
        # 1.3us mid-critical-path before the first real sqrt.
        srt_dummy = wp.tile([1, 1], f32, name="srt_dummy")
        nc.scalar.sqrt(srt_dummy[:], ones_f32[0:1, 0:1])
        # fp16 copy of the index-value rows (exact for these small ints);
        # on the Scalar engine so it does not queue behind the DVE z-chain.
        mgr_h = wp.tile([1, MR_N], f16, name="mgr_h")
        nc.scalar.copy(mgr_h[:], mgr[0:1, MR0:MR0 + MR_N])

        # ---------------- index-row broadcasts (fp16 PE ones-matmuls) -----
        bc1 = pss.tile([64, 450], f32, name="bc1", tag="pssm")
        nc.tensor.matmul(bc1[:], ones_h[:1, 0:64], mgr_h[:1, 0:450],
                         start=True, stop=True)
        bc2 = pss.tile([128, 512], f32, name="bc2", tag="pssm")
        nc.tensor.matmul(bc2[:], ones_h[:1, 0:128], mgr_h[:1, 450:962],
                         start=True, stop=True)
        bc3 = pss.tile([CAP_S, CAP_E1], f32, name="bc3", tag="pssm")
        nc.tensor.matmul(bc3[:], ones_h[:1, 0:CAP_S], mgr_h[:1, 962:1026],
                         start=True, stop=True)
        ds2 = pss.tile([1, CAP_E0], f32, name="ds2", tag="pssm")
        nc.tensor.matmul(ds2[:], ones_rows[0:3, :1], sqr[:],
                         start=True, stop=True)
        dist = tp.tile([1, CAP_E0], f32, name="dist", tag="dist")
        nc.scalar.sqrt(dist[:], ds2[:])

        # selectors (DVE is_equal against per-partition iota); the combined
        # selAB = [x1R-extract | selA | selB] serves x1's three gathers
        # with a single matmul per chunk.
        sel0s = sb([64, CAP_E0], "sel0s")
        iseq(sel0s[:], bc1[:, 0:384], iota_part[0][:64, :1])
        sel0d = sb([64, CAP_E0], "sel0d")
        iseq(sel0d[:], bc2[0:64, 0:384], iota_part[0][:64, :1])
        selE = []
        for t in range(NT0):
            s_ = sb([128, CAP_E1], f"selE{t}")
            iseq(s_[:], bc2[:, 384:448], iota_part[t][:, :1])
            selE.append(s_)
        selAB = sb([CAP_S, 8 + 2 * CAP_E1], "selAB")
        nc.vector.tensor_copy(selAB[:, 0:8], ident[:CAP_S, 0:R_PER])
        iseq(selAB[:, 8:8 + CAP_E1], bc2[0:CAP_S, 448:512],
             iota_part[0][:CAP_S, :1])
        iseq(selAB[:, 8 + CAP_E1:8 + 2 * CAP_E1], bc3[:, 0:CAP_E1],
             iota_part[0][:CAP_S, :1])

        # ---------------- z normalization -------------------------------
        zsq = tp.tile([64, 512], f32, name="zsq", tag="scr")
        zss = wp.tile([64, 1], f32, name="zss")
        nc.vector.tensor_tensor(zsq[:], mz, mz, op=OP.mult)
        nc.vector.tensor_reduce(zss[:], zsq[:], axis=mybir.AxisListType.X,
                                op=OP.add)
        nc.vector.tensor_scalar(zss[:], zss[:], 1.0 / 512.0, 1e-8,
                                OP.mult, OP.add)
        zsr = wp.tile([64, 1], f32, name="zsr")
        nc.scalar.sqrt(zsr[:], zss[:])
        zrin = wp.tile([64, 1], f32, name="zrin")
        nc.vector.reciprocal(zrin[:], zsr[:])
        zbf = sb([64, 512], "zbf")
        nc.vector.tensor_copy(zbf[:], mz)  # raw z, bf16 (norm later)
        selS = sb([64, CAP_S], "selS")
        iseq(selS[:], bc1[:, 384:450], iota_part[0][:64, :1])
        selSS = sb([64, CAP_S], "selSS")
        nc.vector.tensor_scalar_mul(selSS[:], selS[:], zrin[:, :1])

        # ---------------- PE: z transposes ------------------------------
        # f32 transpose of raw z; the psum copy casts to bf16.
        znT = []
        for k in range(4):
            znT.append(peT(mzc[0:64, 128 * k:128 * (k + 1)], 64, 128,
                           f"znT{k}"))

        # laRhs: feature-major rhs [97 used rows, E0] matching laraw layout
        laRhs = sb([97, CAP_E0], "laRhs")
        nc.gpsimd.memset(laRhs[:], 0.0)
        nc.vector.tensor_copy(laRhs[0:3, :], mgr[0:3, GEO_S:GEO_S + CAP_E0])
        nc.vector.tensor_copy(laRhs[32:35, :], mgr[0:3, GEO_D:GEO_D + CAP_E0])
        nc.vector.tensor_copy(laRhs[64:65, :], dist[:])
        nc.vector.tensor_copy(laRhs[96:97, :],
                              ones_f32[:1, :1].to_broadcast([1, CAP_E0]))

        # token-major la[dst] (for the agg tail) and la[S]
        ldst_bf = sb([128, 9], "ldst_bf")
        nc.scalar.copy(ldst_bf[:], mzc[:, CZ_LDST:CZ_LDST + 9])
        laS_bf = sb([CAP_S, 3], "laS_bf")
        nc.scalar.copy(laS_bf[:], mzc[0:CAP_S, CZ_LAS:CZ_LAS + 3])

        # rhs combo tile for the n00 layer
        rhs_n00 = sb([65, CAP_S], "rhs_n00")
        nc.gpsimd.memset(rhs_n00[:], 0.0)
        nc.vector.tensor_copy(rhs_n00[64:65, :],
                              ones_f32[:1, :1].to_broadcast([1, CAP_S]))
        copyT(laS_bf[:], CAP_S, 3, rhs_n00[0:3, :])

        # ---------------- zterm + zgS (PE; needs GA weights) --------------
        def zterm(base, name):
            ps_zt = psb.tile([64, 512], f32, name=f"ps_{name}", tag="psbig")
            for k in range(4):
                nc.tensor.matmul(ps_zt[:], znT[k][:], W(base + k),
                                 start=(k == 0), stop=(k == 3))
            t_ = sb([64, 512], name)
            # z-norm scale folded into the PSUM->SBUF copy (per-z-row),
            # on the Scalar engine (act copy with per-partition scale)
            nc.scalar.activation(t_[:], ps_zt[:], AF.Copy, bias=0.0,
                                 scale=zrin[:, :1])
            return t_

        ztermA = zterm(T_ZSRC, "ztermA")
        ztermB = zterm(T_ZDST, "ztermB")

        zgS = []
        for c in range(4):
            ps = pss.tile([128, CAP_S], f32, name=f"ps_zg{c}", tag="pssm")
            nc.tensor.matmul(ps[:], zbf[:64, 128 * c:128 * (c + 1)], selSS[:],
                             start=True, stop=True)
            t_ = sb([128, CAP_S], f"zgS{c}")
            ps_copy(t_[:], ps[:])
            zgS.append(t_)

        # ---------------- proc-0 edge MLP layer 1 (feature-major) ---------
        # zterm halves first (need only GA); the laraw half closes each
        # group once GB1 lands, so the PE streams through the GA->GB1 gap.
        h0 = []
        h0ps = []
        for c in range(4):
            cs = slice(128 * c, 128 * (c + 1))
            ps = pss.tile([128, CAP_E0], f32, name=f"ps_efp{c}", tag="pssm")
            nc.tensor.matmul(ps[:], ztermA[:64, cs], sel0s[:],
                             start=True, stop=False)
            nc.tensor.matmul(ps[:], ztermB[:64, cs], sel0d[:],
                             start=False, stop=False)
            h0ps.append(ps)
        for c in range(4):
            cs = slice(128 * c, 128 * (c + 1))
            nc.tensor.matmul(h0ps[c][:], wbig[0:97, T_LARAW, cs],
                             laRhs[0:97, :], start=False, stop=True)
        h0 = [sb([128, CAP_E0], f"h0_{c}") for c in range(4)]

        def h0_chunk(t):
            es = slice(128 * t, 128 * (t + 1))
            for c in range(4):
                tt = tp.tile([128, 128], f32, name=f"lh0_{t}_{c}",
                             tag="lrh0")
                nc.scalar.copy(tt[:], h0ps[c][:, es])
                nc.vector.scalar_tensor_tensor(h0[c][:, es], tt[:], 0.2,
                                               h0ps[c][:, es],
                                               op0=OP.mult, op1=OP.max)

        # ---------------- proc-0 edge MLP layer 2 (token-major) -----------
        # msg tile t needs only e-chunk t of each h0[c]; the h0 chunk
        # lrelus are batched per tile and interleaved with the msg
        # emission, so msg starts ~2us earlier AND each msg tile's own
        # lrelu queues ahead of the next h0 batch -- the psb=2 slot
        # recycling never waits behind unrelated h0 stts on the DVE.
        h0_chunk(0)
        msg = []
        for t in range(NT0):
            es = slice(128 * t, 128 * (t + 1))
            ps_m = psb.tile([128, 512], f32, name=f"ps_ef0{t}", tag="psbig")
            brow_mm(ps_m, "e01", 128)
            for k in range(4):
                nc.tensor.matmul(ps_m[:], h0[k][:, es], W(T_W0E1 + k),
                                 start=False, stop=(k == 3))
            if t + 1 < NT0:
                h0_chunk(t + 1)
            m = sb([128, 512], f"msg{t}")
            lrelu(ps_m[:], m[:], s_copy=True)
            msg.append(m)

        # ---------------- node MLP 0 (FEATURE-major) --------------------
        # out[f, s]: N=66 moving rows per matmul instead of 512, and the
        # fm output chunks are directly the lhsT of the token-major n01 --
        # no transposes.  The zn half opens the accumulations with ready
        # inputs, filling the PE drain while the last msg tile's lrelu is
        # still on the DVE.
        # proc-1 ef0 gathers (need only msg + selE)
        ef0g = []
        for c in range(4):
            ps_g = pss.tile([128, CAP_E1], f32, name=f"ps_ef0g{c}", tag="pssm")
            for t in range(NT0):
                nc.tensor.matmul(ps_g[:], msg[t][:, 128 * c:128 * (c + 1)],
                                 selE[t][:], start=(t == 0),
                                 stop=(t == NT0 - 1))
            o = sb([128, CAP_E1], f"ef0g{c}")
            ps_copy(o[:], ps_g[:])
            ef0g.append(o)

        # ---------------- aggregation onto S (feature-major, rin folded) --
        aggT = []
        for c in range(4):
            ps_ag = pss.tile([128, CAP_S], f32, name=f"ps_agg{c}", tag="pssm")
            for t in range(NT0):
                nc.tensor.matmul(ps_ag[:], msg[t][:, 128 * c:128 * (c + 1)],
                                 G0R[:, CAP_S * t:CAP_S * (t + 1)],
                                 start=(t == 0), stop=(t == NT0 - 1))
            t_ = sb([128, CAP_S], f"aggT{c}")
            ps_copy(t_[:], ps_ag[:])
            aggT.append(t_)
        ps_tl = pss.tile([3, CAP_S], f32, name="ps_aggtl", tag="pssm")
        for t in range(NT0):
            nc.tensor.matmul(ps_tl[:], ldst_bf[:, 3 * t:3 * (t + 1)],
                             G0R[:, CAP_S * t:CAP_S * (t + 1)],
                             start=(t == 0), stop=(t == NT0 - 1))
        ps_copy(rhs_n00[32:35, :], ps_tl[:])

        # ---------------- node MLP 0 (FEATURE-major) --------------------
        # out[f, s]: N=66 moving rows per matmul instead of 512, and the
        # fm output chunks are directly the lhsT of the token-major n01 --
        # no transposes.  Groups run fc-sequentially (one open psum bank
        # at a time); each group's lrelu overlaps the next group's mms.
        hnT = [sb([128, CAP_S], f"hnT{c}") for c in range(4)]
        for fc in range(4):
            fs = slice(128 * fc, 128 * (fc + 1))
            ps_nf = pss.tile([128, CAP_S], f32, name=f"ps_n00f{fc}",
                             tag="pssm")
            for c in range(4):
                nc.tensor.matmul(ps_nf[:], wbig[:, T_W0N0Z + c, fs],
                                 zgS[c][:], start=(c == 0), stop=False)
            for c in range(4):
                nc.tensor.matmul(ps_nf[:], wbig[:, T_W0N0A + c, fs],
                                 aggT[c][:], start=False, stop=False)
            nc.tensor.matmul(ps_nf[:], wbig[0:65, T_N00C, fs],
                             rhs_n00[0:65, :], start=False, stop=True)
            # fm lrelu: stage on Scalar, max on DVE
            t_ = tp.tile([128, CAP_S], f32, name=f"lrn0{fc}", tag="lrnf")
            nc.scalar.copy(t_[:], ps_nf[:])
            nc.vector.scalar_tensor_tensor(hnT[fc][:], t_[:], 0.2,
                                           ps_nf[:],
                                           op0=OP.mult, op1=OP.max)

        # n01; e10 is FEATURE-major (N=64 matmuls), fc-sequential after
        # the x1 gathers; the fm outputs are directly the lhsT of
        # token-major e11, and its per-partition bias folds into the fm
        # lrelu staging on the Scalar engine.
        ps = psb.tile([CAP_S, 512], f32, name="ps_n01", tag="psbig")
        brow_mm(ps, "n01", CAP_S)
        for c in range(4):
            nc.tensor.matmul(ps[:], hnT[c][:], W(T_W0N1 + c),
                             start=False, stop=(c == 3))
        x1tok = sb([CAP_S, 512], "x1tok")
        NG = 8 + 2 * CAP_E1
        x1g = [sb([128, NG], f"x1g{c}") for c in range(4)]
        x1R = [x1g[c][:, 0:R_PER] for c in range(4)]

        # e10-fm fc0/fc1 open early with the ready ef0g half; each x1
        # chunk then feeds their A/B matmuls inline, so the PE streams
        # through the x1-gather round-trips instead of idling.
        h1T = [sb([128, CAP_E1], f"h1T{c}") for c in range(4)]
        be10 = mzc[:, CZ_BE10:CZ_BE10 + 4]
        ps_efE = [pss.tile([128, CAP_E1], f32, name=f"ps_e10f{fc}",
                           tag="pssm") for fc in range(2)]
        for fc in range(2):
            for c in range(4):
                nc.tensor.matmul(ps_efE[fc][:],
                                 wbig8[:, T8_W1E0 + 8 + c,
                                       128 * fc:128 * (fc + 1)],
                                 ef0g[c][:], start=(c == 0), stop=False)

        # 256-col lrelu chunks halve the PE<->DVE<->Scalar round-trips;
        # each chunk feeds two 128-col [x1R | x1(srcE1) | x1(dstE1)]
        # gather matmuls plus fc0's A/B matmuls.
        for c2 in range(2):
            cs = slice(256 * c2, 256 * (c2 + 1))
            t = tp.tile([CAP_S, 256], f32, name=f"lrx1{c2}", tag="lrx1")
            if c2 == 0:
                nc.vector.tensor_copy(t[:], ps[:, cs])
            else:
                nc.scalar.copy(t[:], ps[:, cs])
            nc.vector.scalar_tensor_tensor(x1tok[:, cs], t[:], 0.2,
                                           ps[:, cs],
                                           op0=OP.mult, op1=OP.max)
            for h in range(2):
                c = 2 * c2 + h
                ps_ = pss.tile([128, NG], f32, name=f"ps_x1g{c}",
                               tag="pssm")
                nc.tensor.matmul(ps_[:],
                                 x1tok[:, 128 * c:128 * (c + 1)],
                                 selAB[:], start=True, stop=True)
                nc.vector.tensor_copy(x1g[c][:], ps_[:])
                for fc in range(2):
                    fs0 = slice(128 * fc, 128 * (fc + 1))
                    nc.tensor.matmul(ps_efE[fc][:],
                                     wbig8[:, T8_W1E0 + c, fs0],
                                     x1g[c][:, 8:8 + CAP_E1],
                                     start=False, stop=False)
                    nc.tensor.matmul(ps_efE[fc][:],
                                     wbig8[:, T8_W1E0 + 4 + c, fs0],
                                     x1g[c][:, 8 + CAP_E1:NG],
                                     start=False, stop=(c == 3))
        for fc in range(2):
            t_ = tp.tile([128, CAP_E1], f32, name=f"lre1f{fc}", tag="lref")
            nc.scalar.activation(t_[:], ps_efE[fc][:], AF.Identity,
                                 bias=be10[:, fc:fc + 1], scale=1.0 / F8S)
            nc.vector.scalar_tensor_tensor(h1T[fc][:], t_[:], 0.2, t_[:],
                                           op0=OP.mult, op1=OP.max)
        for fc in range(2, 4):
            fs = slice(128 * fc, 128 * (fc + 1))
            ps_ef = pss.tile([128, CAP_E1], f32, name=f"ps_e10f{fc}",
                             tag="pssm")
            for c in range(4):
                nc.tensor.matmul(ps_ef[:], wbig8[:, T8_W1E0 + c, fs],
                                 x1g[c][:, 8:8 + CAP_E1],
                                 start=(c == 0), stop=False)
                nc.tensor.matmul(ps_ef[:], wbig8[:, T8_W1E0 + 4 + c, fs],
                                 x1g[c][:, 8 + CAP_E1:NG],
                                 start=False, stop=False)
            for c in range(4):
                nc.tensor.matmul(ps_ef[:], wbig8[:, T8_W1E0 + 8 + c, fs],
                                 ef0g[c][:], start=False, stop=(c == 3))
            # fm lrelu with per-partition bias: t = psum/F8S + b, max on DVE
            t_ = tp.tile([128, CAP_E1], f32, name=f"lre1{fc}", tag="lref")
            nc.scalar.activation(t_[:], ps_ef[:], AF.Identity,
                                 bias=be10[:, fc:fc + 1], scale=1.0 / F8S)
            nc.vector.scalar_tensor_tensor(h1T[fc][:], t_[:], 0.2, t_[:],
                                           op0=OP.mult, op1=OP.max)

        # e11 (token-major; lhsT = fm h1 chunks directly); each chunk
        # feeds the R-aggregation feature-major (rin1 folded into G1R) and
        # then its 4 fm n10 matmuls -- no transposes anywhere.
        msg1 = sb([CAP_E1, 512], "msg1")
        ps_e11 = psb.tile([CAP_E1, 512], f32, name="ps_e11", tag="psbig")
        brow_mm(ps_e11, "e11", CAP_E1)
        for c in range(4):
            nc.tensor.matmul(ps_e11[:], h1T[c][:], W(T_W1E1 + c),
                             start=False, stop=(c == 3))
        agg1T = [sb([128, R_PER], f"agg1T{c}") for c in range(4)]
        # n10-fm fc0/fc1 open early with the ready x1R half; each agg1T
        # then feeds their matmuls inline, filling e11's lrelu round-trips.
        bn10 = mzc[:, CZ_BN10:CZ_BN10 + 4]
        ps_ntE = [pss.tile([128, R_PER], f32, name=f"ps_n10f{fc}",
                           tag="pssm") for fc in range(2)]
        for fc in range(2):
            for c in range(4):
                nc.tensor.matmul(ps_ntE[fc][:],
                                 wbig[:, T_W1N0 + c,
                                      128 * fc:128 * (fc + 1)],
                                 x1g[c][:, 0:R_PER],
                                 start=(c == 0), stop=False)
        for c2 in range(2):
            cs = slice(256 * c2, 256 * (c2 + 1))
            t = tp.tile([CAP_E1, 256], f32, name=f"lre11{c2}", tag="lre2")
            if c2 == 0:
                nc.vector.tensor_copy(t[:], ps_e11[:, cs])
            else:
                nc.scalar.copy(t[:], ps_e11[:, cs])
            nc.vector.scalar_tensor_tensor(msg1[:, cs], t[:], 0.2,
                                           ps_e11[:, cs],
                                           op0=OP.mult, op1=OP.max)
            for h in range(2):
                c = 2 * c2 + h
                ps_ = pss.tile([128, R_PER], f32, name=f"ps_ag1{c}",
                               tag="pssm")
                nc.tensor.matmul(ps_[:],
                                 msg1[:, 128 * c:128 * (c + 1)],
                                 G1R, start=True, stop=True)
                nc.vector.tensor_copy(agg1T[c][:], ps_[:])
                for fc in range(2):
                    nc.tensor.matmul(ps_ntE[fc][:],
                                     wbig[:, T_W1N0 + 4 + c,
                                          128 * fc:128 * (fc + 1)],
                                     agg1T[c][:], start=False,
                                     stop=(c == 3))

        # ---------------- final node MLPs (fm n10 -> token n11) -----------
        hfT = [sb([128, R_PER], f"hfT{c}") for c in range(4)]
        ps_n11 = psb.tile([R_PER, 512], f32, name="ps_n11", tag="psbig")
        brow_mm(ps_n11, "n11", R_PER)
        for fc in range(2):
            t_ = tp.tile([128, R_PER], f32, name=f"lrnf{fc}", tag="lrn1")
            nc.scalar.activation(t_[:], ps_ntE[fc][:], AF.Identity,
                                 bias=bn10[:, fc:fc + 1], scale=1.0)
            nc.vector.scalar_tensor_tensor(hfT[fc][:], t_[:], 0.2, t_[:],
                                           op0=OP.mult, op1=OP.max)
            nc.tensor.matmul(ps_n11[:], hfT[fc][:], W(T_W1N1 + fc),
                             start=False, stop=False)
        for fc in range(2, 4):
            fs = slice(128 * fc, 128 * (fc + 1))
            ps_nt = pss.tile([128, R_PER], f32, name=f"ps_n10f{fc}",
                             tag="pssm")
            for c in range(4):
                nc.tensor.matmul(ps_nt[:], wbig[:, T_W1N0 + c, fs],
                                 x1g[c][:, 0:R_PER],
                                 start=(c == 0), stop=False)
            for c in range(4):
                nc.tensor.matmul(ps_nt[:], wbig[:, T_W1N0 + 4 + c, fs],
                                 agg1T[c][:], start=False, stop=(c == 3))
            t_ = tp.tile([128, R_PER], f32, name=f"lrnf{fc}", tag="lrn1")
            nc.scalar.activation(t_[:], ps_nt[:], AF.Identity,
                                 bias=bn10[:, fc:fc + 1], scale=1.0)
            nc.vector.scalar_tensor_tensor(hfT[fc][:], t_[:], 0.2, t_[:],
                                           op0=OP.mult, op1=OP.max)
            nc.tensor.matmul(ps_n11[:], hfT[fc][:], W(T_W1N1 + fc),
                             start=False, stop=(fc == 3))
        wstok = wp.tile([R_PER, 512], f32, name="wstok")
        lrelu(ps_n11[:], wstok[:])

        # out on the scalar ring: its config issues early (the sync ring's
        # sequencer is blocked by chain waits until ~27us, and keeping the
        # out there delays the sync ring's teardown drain to ~47us).
        nc.scalar.dma_start(out_d[:, :], wstok[:, :])

        if DEBUG_DUMPS:
            for nm, t_ in [("ztermA", ztermA), ("ztermB", ztermB),
                           ("h0_0", h0[0]), ("msg0", msg[0]),
                           ("aggT0", aggT[0]), ("hnT0", hnT[0]),
                           ("x1tok", x1tok), ("h1T0", h1T[0]),
                           ("msg1", msg1), ("hfT0", hfT[0]),
                           ("laRhs", laRhs),
                           ("zgS0", zgS[0]), ("rhs_n00", rhs_n00),
                           ("sel0s", sel0s), ("agg1T0", agg1T[0]),
                           ("ef0g0", ef0g[0]), ("x1g0", x1g[0])]:
                shp = list(t_.shape)
                dd = nc.dram_tensor(f"dbg_{nm}", shp, t_.dtype,
                                    kind="ExternalOutput")
                nc.sync.dma_start(dd[:, :], t_[:, :])

    nc.finalize()
    return nc


_PROG_CACHE = {}


def _get_program():
    key = (CAP_E0, CAP_S, CAP_E1)
    if key not in _PROG_CACHE:
        _PROG_CACHE[key] = _build_program()
    return _PROG_CACHE[key]


def _pad(a, n, fill):
    out = np.full((n,), fill, dtype=np.float32)
    out[:len(a)] = a.astype(np.float32)
    return out


def _host_weights(inputs):
    """Pack all FC weights (transposed, gain*sqrt2 pre-folded) + biases
    into one [NT*128, 512] bf16 tensor of K-tiles."""
    f = np.float32
    s = SQ2

    def T(name):
        return np.ascontiguousarray(np.asarray(inputs[name], f).T)

    w0e0T, w0e1T = T("p0_ew0"), T("p0_ew1")
    w0n0T, w0n1T = T("p0_nw0"), T("p0_nw1")
    w1e0T, w1e1T = T("p1_ew0"), T("p1_ew1")
    w1n0T, w1n1T = T("p1_nw0"), T("p1_nw1")

    def bias(name):
        return np.asarray(inputs[name], f)

    wpk = np.zeros((NT * 128, 512), f)

    def put(idx, rows):
        wpk[idx * 128: idx * 128 + rows.shape[0]] = rows

    put(T_ZSRC, w0e0T[0:512] * (G_E00 * s))
    put(T_ZDST, w0e0T[515:1027] * (G_E00 * s))
    for key, bname in [("e01", "p0_eb1"), ("n01", "p0_nb1"),
                       ("e10", "p1_eb0"), ("e11", "p1_eb1"),
                       ("n10", "p1_nb0"), ("n11", "p1_nb1")]:
        tidx, pbase = BROW_SLOT[key]
        bsc = F8S if key == "e10" else 1.0
        wpk[tidx * 128 + pbase] = bias(bname) * (LR * s * bsc)
    # rel = la[dst]-la[src] folds into the src/dst la blocks:
    #   src rows get (laA - w_rel), dst rows get (laB + w_rel)
    laraw = np.zeros((128, 512), f)
    laraw[0:3] = (w0e0T[512:515] - w0e0T[1030:1033]) * (G_E00 * s)
    laraw[32:35] = (w0e0T[1027:1030] + w0e0T[1030:1033]) * (G_E00 * s)
    laraw[64:65] = w0e0T[1033:1034] * (G_E00 * s)  # dist weight
    laraw[96] = bias("p0_eb0") * (LR * s)
    put(T_LARAW, laraw)
    put(T_W0E1, w0e1T * (G_E01 * s))
    put(T_W0N0Z, w0n0T[0:512] * (G_N00 * s))
    # n00 input dims: 0:512 zn | 512:515 la | 515:518 la_dst-mean | 518:1030
    # ef-mean.  aggT holds the ef-mean block, rhs_n00[32:35] the la_dst-mean.
    put(T_W0N0A, w0n0T[518:1030] * (G_N00 * s))
    comb = np.zeros((128, 512), f)
    comb[0:3] = w0n0T[512:515] * (G_N00 * s)    # la features of x
    comb[32:35] = w0n0T[515:518] * (G_N00 * s)  # la_dst-mean
    comb[64] = bias("p0_nb0") * (LR * s)
    put(T_N00C, comb)
    put(T_W0N1, w0n1T * (G_N01 * s))
    put(T_W1E1, w1e1T * (G_E11 * s))
    put(T_W1N0, w1n0T * (G_N10 * s))
    put(T_W1N1, w1n1T * (G_N11 * s))
    wpk8 = np.zeros((NT8 * 128, 512), f)
    wpk8[T8_W1E0 * 128:(T8_W1E0 + 12) * 128] = w1e0T * (G_E10 * s * F8S)
    wpk8 = wpk8.reshape(NT8 // 4, 4, 128, 512).transpose(0, 2, 1, 3)
    wpk8 = np.ascontiguousarray(wpk8.reshape(NT8 * 128, 512))
    wpk8 = np.ascontiguousarray(wpk8.astype(ml_dtypes.float8_e4m3))
    # pair-interleave rows: tile pair q -> rows (q*128+p)*2+j
    wpk = wpk.reshape(NT // 2, 2, 128, 512).transpose(0, 2, 1, 3)
    wpk = np.ascontiguousarray(wpk.reshape(NT * 128, 512))
    return np.ascontiguousarray(wpk.astype(ml_dtypes.bfloat16)), wpk8


def _core_meta(z, la, src, dst, c, bias_fm):
    """Per-core metadata tensors (integer index-set construction + row
    gathers of input data + 1/count fold; no arithmetic on tensor values)."""
    Rc = (np.arange(R_PER, dtype=np.int64) + c * R_PER) * NV
    E1 = np.nonzero(np.isin(dst, Rc))[0]
    others = np.setdiff1d(np.unique(src[E1]), Rc)
    S = np.concatenate([Rc, others])
    assert len(E1) <= CAP_E1 and len(S) <= CAP_S, (len(E1), len(S))
    slot = np.full(16000, -1, np.int64)
    slot[S] = np.arange(len(S))
    E0 = np.nonzero(slot[dst] >= 0)[0]
    assert len(E0) <= CAP_E0, len(E0)
    pos = np.full(src.shape[0], -1, np.int64)
    pos[E0] = np.arange(len(E0))
    e0s, e0d = src[E0], dst[E0]
    e1s, e1d = src[E1], dst[E1]

    def gat(idx, n):
        out = np.zeros((n, 3), np.float32)
        out[:len(idx)] = la[idx]
        return out

    # rin-folded one-hot gather matrices (bf16, bit-packed into f32 cols)
    cnt0 = np.bincount(slot[e0d].astype(np.int64), minlength=CAP_S)[:CAP_S]
    rin0 = (1.0 / np.maximum(cnt0, 1)).astype(np.float32)
    sig0 = _pad(slot[e0d], CAP_E0, -1).astype(np.int64)
    G0R = np.zeros((128, NT0 * CAP_S), np.float32)
    for t in range(NT0):
        blk = sig0[128 * t:128 * (t + 1)]
        for e in range(128):
            if blk[e] >= 0:
                G0R[e, CAP_S * t + blk[e]] = rin0[blk[e]]
    cnt1 = np.bincount(slot[e1d].astype(np.int64), minlength=R_PER)[:R_PER]
    rin1 = (1.0 / np.maximum(cnt1, 1)).astype(np.float32)
    G1R = np.zeros((CAP_E1, R_PER), np.float32)
    for e in range(len(E1)):
        G1R[e, slot[e1d[e]]] = rin1[slot[e1d[e]]]

    def pack_bf16(a, rows):
        b = np.zeros((rows, a.shape[1]), ml_dtypes.bfloat16)
        b[:a.shape[0]] = a.astype(ml_dtypes.bfloat16)
        if b.shape[1] % 2:
            b = np.concatenate(
                [b, np.zeros((rows, 1), ml_dtypes.bfloat16)], axis=1)
        return np.ascontiguousarray(b).view(np.float32)

    mzc = np.zeros((128, MZC_F), np.float32)
    mzc[0:64, 0:512] = z
    la_d = gat(e0d, CAP_E0).reshape(NT0, 128, 3)
    for t in range(NT0):
        mzc[:, CZ_LDST + 3 * t:CZ_LDST + 3 * (t + 1)] = la_d[t]
    mzc[0:CAP_S, CZ_LAS:CZ_LAS + 3] = gat(S, CAP_S)
    mzc[0:CAP_E1, CZ_G1R:CZ_G1R + 4] = pack_bf16(G1R, CAP_E1)
    mzc[:, CZ_G0R:CZ_G0R + 99] = pack_bf16(G0R, 128)
    mzc[:, CZ_BE10:CZ_BE10 + 12] = bias_fm

    mgr = np.zeros((3, MGR_F), np.float32)
    mgr[0:3, GEO_S:GEO_S + CAP_E0] = gat(e0s, CAP_E0).T
    mgr[0:3, GEO_D:GEO_D + CAP_E0] = gat(e0d, CAP_E0).T
    mrow = np.zeros(MR_N, np.float32)
    mrow[MR_E0GS:MR_E0GS + CAP_E0] = _pad(e0s % B, CAP_E0, -1)
    mrow[MR_SSEL:MR_SSEL + CAP_S] = _pad(S % B, CAP_S, -1)
    mrow[MR_E0GD:MR_E0GD + CAP_E0] = _pad(e0d % B, CAP_E0, -1)
    mrow[MR_E1POS:MR_E1POS + CAP_E1] = _pad(pos[E1], CAP_E1, -1)
    mrow[MR_E1SRC:MR_E1SRC + CAP_E1] = _pad(slot[e1s], CAP_E1, -1)
    mrow[MR_E1DST:MR_E1DST + CAP_E1] = _pad(slot[e1d], CAP_E1, -1)
    mgr[0, MR0:MR0 + MR_N] = mrow

    return {"mzc": mzc,
            "mgr": np.ascontiguousarray(mgr.reshape(24, MGR_F // 8))}


def make_in_maps(inputs):
    ei = np.asarray(inputs["edge_index"])
    src, dst = ei[0].astype(np.int64), ei[1].astype(np.int64)
    z = np.ascontiguousarray(np.asarray(inputs["z"], np.float32))
    la = np.ascontiguousarray(np.asarray(inputs["look_ats"], np.float32))
    wpk, wpk8 = _host_weights(inputs)
    s = np.sqrt(2.0, dtype=np.float32)
    bias_fm = np.stack([
        np.asarray(inputs["p1_eb0"], np.float32).reshape(4, 128).T,
        np.asarray(inputs["p1_nb0"], np.float32).reshape(4, 128).T,
        np.asarray(inputs["p1_nb1"], np.float32).reshape(4, 128).T,
    ], axis=1).reshape(128, 12) * (LR * s)
    return [dict(wpack=wpk, wpack8=wpk8,
                 **_core_meta(z, la, src, dst, c, bias_fm))
            for c in range(N_CORES)]


def kernel(**inputs):
    nc = _get_program()
    in_maps = make_in_maps(inputs)
    res = run_bass_kernel_spmd(nc, in_maps, core_ids=list(range(N_CORES)))
    ws = np.concatenate([res.results[c]["out"] for c in range(N_CORES)],
                        axis=0).astype(np.float32)
    return np.ascontiguousarray(
        np.broadcast_to(ws[:, None, :], (B, 14, D))).astype(np.float32)


# revision 51
# speedup vs baseline: 1.0528x; 1.0528x over previous
"""Trainium2 Bass kernel for nn_LookAtMappingNetwork (gnn_message_passing).

Strategy
--------
The module's output only reads the final node features at rows R = {i*250 :
i in 0..63} (``ws = x[::250]``).  Working backwards through the two message
-passing processors, only a small data-dependent subset of edges/nodes can
influence those rows, for ANY edge_index:

    E1 = edges with dst in R          (<= 58 per core)  -> proc-1 edge MLP
    S  = R  U  src[E1]                (<= 65 per core)  -> rows where x1 needed
    E0 = edges with dst in S          (<= 375 per core) -> proc-0 edge MLP

Segment-mean counts stay exact because E0/E1 contain ALL edges landing on
S/R.  Everything else the reference computes is dead code.  Each of the 8
cores handles 8 output rows (its R_c) fully independently.

Performance layout
------------------
* All FC weights are transposed, pre-scaled by (lr/sqrt(fan_in))*sqrt(2)
  and packed host-side into ONE bf16 tensor of 128-row K-tiles (pair-
  interleaved in DRAM so each DMA line is a contiguous 2 KiB run).  Since
  leaky_relu commutes with positive scaling, each layer's activation
  collapses to copy+max on Scalar/DVE with zero extra scalar work.
* Weight DMA is chained on the sync ring in first-use order via WAW
  overlaps on late-consumed tiles (GA -> GB1 -> GB2 -> GC), so each layer's
  weights land just-in-time while compute streams.
* Metadata rides in just TWO tensors (each dma_start costs ~1.3us of
  serial ring latency): mzc [128 x F] carries z + per-partition values +
  bf16 gather matrices bit-packed into f32 columns; mgr [3 x F] carries
  feature-major la geometry + all index VALUE rows.  Index rows are
  broadcast on-chip with fp16 one-row PE matmuls (exact for these small
  ints, 4x faster than f32) and compared against iota on the DVE.
* Mean-aggregation gather matrices G0R/G1R ship host-side with 1/count
  pre-folded (bf16): aggregation becomes plain feature-major matmuls --
  no token-major agg psum, no DVE rescale, no PE transposes.
* The back half (n00 / e10 / n10) is FEATURE-major: matmul cost is the
  moving-row count, so N drops from 512 to 66/64/8 per matmul (30-60ns
  chained), the fm lrelu chunks are directly the lhsT of the next
  token-major layer (zero transposes), and per-partition biases ride the
  scalar-engine activation staging.  Only layers feeding an aggregation
  or gather (msg, n01, e11) stay token-major, since those contractions
  need tokens on the partition axis.
* PSUM accumulation groups are fc-sequential (one open group per 2 KiB
  zero-region/bank); leaky-relu runs as copy+max staged across
  Scalar/DVE in 256-col chunks with the next layer's consumers emitted
  inline.
* CAP_S=66 / CAP_E1=64 (actual maxima 65/58) halve the mid-kernel token
  dimensions vs the padded-to-128 v1.
* w1e0 (12 tiles) ships as fp8_e4m3 (scaled 2^6; the e10 lrelu rescales by
  2^-6 for free) on the scalar ring.
* Output is written un-replicated [8, 512]; the x14 ws broadcast happens
  on the host.
"""

import math

import ml_dtypes
import numpy as np

import concourse.bacc as bacc
import concourse.bass as bass
import concourse.mybir as mybir
import concourse.tile as tile
from concourse.bass_utils import run_bass_kernel_spmd
from concourse.masks import make_identity

f32 = mybir.dt.float32
fr = mybir.dt.bfloat16
f16 = mybir.dt.float16
i32 = mybir.dt.int32
AF = mybir.ActivationFunctionType
OP = mybir.AluOpType

NV = 250
B = 64
D = 512
LR = 0.01
SQ2 = math.sqrt(2.0)
N_CORES = 8
R_PER = B // N_CORES  # output rows per core

CAP_E0 = 384
CAP_S = 66
CAP_E1 = 64
NT0 = CAP_E0 // 128

G_E00 = LR / math.sqrt(1034.0)
G_E01 = LR / math.sqrt(512.0)
G_N00 = LR / math.sqrt(1030.0)
G_N01 = LR / math.sqrt(512.0)
G_E10 = LR / math.sqrt(1536.0)
G_E11 = LR / math.sqrt(512.0)
G_N10 = LR / math.sqrt(1024.0)
G_N11 = LR / math.sqrt(512.0)

# ---- packed weight tile indices (each tile = [128, 512] bf16) ----
T_ZSRC = 0     # 4 tiles: w0e0^T rows 0:512    (z of src)
T_ZDST = 4     # 4 tiles: w0e0^T rows 515:1027 (z of dst)
# pads 8, 9 (G0b overlap)
T_LARAW = 10   # 0:3 laA-rel | 32:35 laB+rel | 64:65 wd | 96:97 b_e00
T_BROWS_E = 11  # bias rows for e01@0 / n01@32 / e10@64
T_W0E1 = 12    # 4
T_BROWS_L = 16  # bias rows for e11@0 / n10@32 / n11@64 (late-consumed)
T_N00C = 17    # 0:3 la | 32:35 la_dst-mean | 64:65 b_n00 (late-consumed)
T_W0N0Z = 18   # 4: w0n0^T rows 0:512
T_W0N0A = 22   # 4: w0n0^T rows 518:1030 (ef-mean block)
T_W0N1 = 26    # 4
T_W1E1 = 30    # 4
T_W1N0 = 34    # 8
T_W1N1 = 42    # 4
NT = 46
# chain groups: G0a=[0:10) -> G0b=[8:18) -> G2=[16:34) -> G3=[32:46)
#   G2 rewrites 16,17 (BROWS_L, N00C: consumed at n00/e11/n10/n11)
#   G3 rewrites 32,33 (W1E1 tiles 2,3: consumed at e11, ~when G3 lands)
# The fp8 pack is issued on the sync ring between G2 and G3: its config
# waits behind G2's chain wait, so it streams in the post-G2 window
# without stealing bandwidth from the critical bf16 stream.

T8_W1E0 = 0    # 12 tiles (fp8 pack)
NT8 = 12
F8S = 64.0

# brow key -> (tile, partition base): matmul bases must be 0/32/64
BROW_SLOT = {"e01": (T_BROWS_E, 0), "n01": (T_BROWS_E, 32),
             "e10": (T_BROWS_E, 64), "e11": (T_BROWS_L, 0),
             "n10": (T_BROWS_L, 32), "n11": (T_BROWS_L, 64)}

# ---- mzc [128, MZC_F] f32: z + per-partition values + packed bf16 ----
# rows 0:64 cols 0:512 = z
CZ_LDST = 512   # 9 cols: look_ats[e0 dst] token-major, 3 per e-tile
CZ_LAS = 521    # 3 cols: look_ats[S] token-major, rows 0:CAP_S
CZ_G1R = 524    # 4 f32 = 8 bf16: G1R [CAP_E1 rows, R_PER] (rin1 folded)
CZ_G0R = 528    # 99 f32 = 198 bf16: G0R [128 rows, 66*3] (rin0 folded)
CZ_BE10 = 627   # 4 cols: p1_eb0 * LR*sqrt2, feature-major [128, 4]
CZ_BN10 = 631   # 4 cols: p1_nb0 * LR*sqrt2, feature-major [128, 4]
CZ_BN11 = 635   # 4 cols: p1_nb1 * LR*sqrt2, feature-major [128, 4]
MZC_F = 639

# ---- mgr [3, MGR_F] f32: feature-major geometry + index value rows ----
GEO_S = 0       # [0:3, 0:384] la[e0 src]^T
GEO_D = 384     # [0:3, 384:768] la[e0 dst]^T
MR0 = 768       # row 0 only: packed index-value rows (see MR_* below)
MGR_F = 1800  # = 8 * 225; shipped as [24, 225] for DMA parallelism
# offsets within the fp16-cast copy mgr_h [1, 1026]
MR_E0GS = 0     # 384: e0 src % B
MR_SSEL = 384   # 66: S % B
MR_E0GD = 450   # 384: e0 dst % B
MR_E1POS = 834  # 64: e1 -> position in E0
MR_E1SRC = 898  # 64
MR_E1DST = 962  # 64
MR_N = 1026

DEBUG_DUMPS = False  # set True to add dbg_* DRAM dumps of intermediates


def _build_program():
    nc = bacc.Bacc("TRN2", target_bir_lowering=False, debug=False,
                   enable_asserts=False, num_devices=N_CORES)

    wpack_d = nc.dram_tensor("wpack", [NT * 128, 512], fr, kind="ExternalInput")
    wpack8_d = nc.dram_tensor("wpack8", [NT8 * 128, 512], mybir.dt.float8e4,
                              kind="ExternalInput")
    mzc_d = nc.dram_tensor("mzc", [128, MZC_F], f32, kind="ExternalInput")
    mgr_d = nc.dram_tensor("mgr", [24, MGR_F // 8], f32,
                           kind="ExternalInput")
    out_d = nc.dram_tensor("out", [R_PER, D], f32, kind="ExternalOutput")

    with tile.TileContext(nc) as tc, \
            tc.tile_pool(name="w", bufs=1) as wp, \
            tc.tile_pool(name="tmp", bufs=8) as tp, \
            tc.tile_pool(name="psb", bufs=2, space="PSUM") as psb, \
            tc.tile_pool(name="pss", bufs=6, space="PSUM") as pss:

        # ---------------- input DMAs ---------------------------------
        # scalar ring: meta first (small, unblocks the front-end), then fp8.
        mgr = wp.tile([3, MGR_F], f32, name="mgr")
        nc.scalar.dma_start(
            mgr[:, :].rearrange("p (k c) -> p k c", k=8),
            mgr_d[:, :].rearrange("(p k) c -> p k c", p=3))
        mzc = wp.tile([128, MZC_F], f32, name="mzc")
        nc.scalar.dma_start(mzc[:], mzc_d[:, :])

        wbig = wp.tile([128, NT, 512], fr, name="wbig")
        wbig8 = wp.tile([128, NT8, 512], mybir.dt.float8e4, name="wbig8")

        def wload(eng, a, b_):
            eng.dma_start(
                wbig[:, a:b_, :].rearrange("p (q j) d -> p q j d", j=2),
                wpack_d[128 * a:128 * b_, :].rearrange(
                    "(q p j) d -> p q j d", p=128, j=2))

        # Chained on the sync ring in first-use order via WAW overlaps.
        wload(nc.sync, 0, 10)          # z (+pads)
        wload(nc.sync, 8, 18)          # laraw, brows_e, w0e1, brows_l, n00c
        wload(nc.sync, 16, 34)         # w0n0, w0n1, w1e1
        wload(nc.sync, 32, 46)         # w1n0, w1n1
        nc.sync.dma_start(
            wbig8[:, :, :].rearrange("p (q j) d -> p q j d", j=4),
            wpack8_d[:, :].rearrange("(q p j) d -> p q j d", p=128, j=4))


        def W8(i):
            return wbig8[:, i, :]

        def W(i):
            return wbig[:, i, :]

        # bf16 gather matrices bit-packed in mzc's f32 columns
        G0R = mzc[:, CZ_G0R:CZ_G0R + 99].bitcast(fr)     # [128, 198]
        G1R = mzc[0:CAP_E1, CZ_G1R:CZ_G1R + 4].bitcast(fr)  # [64, 8]

        # ---------------- constants ----------------
        ident_f = wp.tile([128, 128], f32, name="ident_f")
        make_identity(nc, ident_f[:])
        ident = wp.tile([128, 128], fr, name="ident")
        nc.vector.tensor_copy(ident[:], ident_f[:])
        idents = {fr: ident, f32: ident_f}
        ones_f32 = wp.tile([128, 1], f32, name="ones_f32")
        nc.gpsimd.memset(ones_f32[:], 1.0)
        ones_h = wp.tile([1, 128], f16, name="ones_h")
        nc.gpsimd.memset(ones_h[:], 1.0)
        iota_part = []
        for t in range(NT0):
            it = wp.tile([128, 1], f32, name=f"iota_part{t}")
            nc.gpsimd.iota(it[:], pattern=[[1, 1]], base=128 * t,
                           channel_multiplier=1,
                           allow_small_or_imprecise_dtypes=True)
            iota_part.append(it)
        # ones rows at partition bases 0/32/64 (for bias-row matmuls)
        ones_rows = wp.tile([65, 128], fr, name="ones_rows")
        nc.vector.tensor_copy(ones_rows[:], ones_f32[:65, :1].to_broadcast([65, 128]))

        _uid = [0]

        def uid():
            _uid[0] += 1
            return _uid[0]

        def sb(shape, name):
            return wp.tile(shape, fr, name=name)

        _cp = [0]

        def ps_copy(dst_ap, src_ap):
            """PSUM->SBUF copy, alternating Vector/Scalar engines."""
            _cp[0] += 1
            if _cp[0] % 2 == 0:
                nc.vector.tensor_copy(dst_ap, src_ap)
            else:
                nc.scalar.copy(dst_ap, src_ap)

        def copyT(src_ap, p, f, dst_ap):
            """PE transpose src [p, f] -> existing sbuf dst_ap [f, p]."""
            sdt = src_ap.dtype
            ps = pss.tile([f, p], sdt, name=f"psT{uid()}", tag="pssm")
            nc.tensor.transpose(ps[:], src_ap, idents[sdt][:p, :p])
            ps_copy(dst_ap, ps[:])

        def peT(src_ap, p, f, name):
            dst = sb([f, p], name)
            copyT(src_ap, p, f, dst[:])
            return dst

        def brow_mm(ps_t, key, p):
            tidx, pbase = BROW_SLOT[key]
            nc.tensor.matmul(ps_t[:], ones_rows[pbase:pbase + 1, :p],
                             wbig[pbase:pbase + 1, tidx, :],
                             start=True, stop=False)

        def lrelu(ps_ap, out_ap, s_copy=False):
            """out = leaky_relu(psum, 0.2) -- gain pre-folded into weights.
            (The DVE cannot read two PSUM operands, so stage through SBUF.)"""
            p, n = ps_ap.shape
            t = tp.tile([p, n], f32, name=f"lr{uid()}", tag=f"lr{p}_{n}")
            if s_copy:
                nc.scalar.copy(t[:], ps_ap)
            else:
                nc.vector.tensor_copy(t[:], ps_ap)
            nc.vector.scalar_tensor_tensor(out_ap, t[:], 0.2, ps_ap,
                                           op0=OP.mult, op1=OP.max)

        def lrelu_chunk(ps_t, out_t, p, consume, scale=None):
            """Chunked lrelu over 4 x 128 output columns; consume(c, out_ap)
            emits the chunk's consumers right away so the PE restarts while
            later chunks are still on the DVE."""
            for c in range(4):
                cs = slice(128 * c, 128 * (c + 1))
                t = tp.tile([p, 128], f32, name=f"lrc{uid()}", tag=f"lrc{p}")
                if scale is None:
                    if c == 0:
                        nc.vector.tensor_copy(t[:], ps_t[:, cs])
                    else:
                        nc.scalar.copy(t[:], ps_t[:, cs])
                    nc.vector.scalar_tensor_tensor(out_t[:, cs], t[:], 0.2,
                                                   ps_t[:, cs],
                                                   op0=OP.mult, op1=OP.max)
                else:
                    if c == 0:
                        nc.vector.tensor_scalar_mul(t[:], ps_t[:, cs], scale)
                    else:
                        nc.scalar.activation(t[:], ps_t[:, cs], AF.Identity,
                                             bias=0.0, scale=scale)
                    nc.vector.scalar_tensor_tensor(out_t[:, cs], t[:], 0.2,
                                                   t[:],
                                                   op0=OP.mult, op1=OP.max)
                consume(c, out_t[:, cs])

        def iseq(out_ap, in_ap, iota_t):
            nc.vector.tensor_scalar(out_ap, in_ap, iota_t, None, OP.is_equal)

        mz = mzc[0:64, 0:512]

        # geometry first on the DVE (mgr lands well before mzc)
        rel = tp.tile([3, CAP_E0], f32, name="rel", tag="rel")
        nc.vector.tensor_tensor(rel[:], mgr[0:3, GEO_D:GEO_D + CAP_E0],
                                mgr[0:3, GEO_S:GEO_S + CAP_E0],
                                op=OP.subtract)
        sqr = sb([3, CAP_E0], "sqr")
        nc.vector.tensor_tensor(sqr[:], rel[:], rel[:], op=OP.mult)

        # dummy sqrt: forces the sqrt_and_friends act table (which also
        # serves Copy/Identity) to load ONCE in the preamble instead of
        # 1.3us mid-critical-path before the first real sqrt.
        srt_dummy = wp.tile([1, 1], f32, name="srt_dummy")
        nc.scalar.sqrt(srt_dummy[:], ones_f32[0:1, 0:1])
        # fp16 copy of the index-value rows (exact for these small ints);
        # on the Scalar engine so it does not queue behind the DVE z-chain.
        mgr_h = wp.tile([1, MR_N], f16, name="mgr_h")
        nc.scalar.copy(mgr_h[:], mgr[0:1, MR0:MR0 + MR_N])

        # ---------------- index-row broadcasts (fp16 PE ones-matmuls) -----
        bc1 = pss.tile([64, 450], f32, name="bc1", tag="pssm")
        nc.tensor.matmul(bc1[:], ones_h[:1, 0:64], mgr_h[:1, 0:450],
                         start=True, stop=True)
        bc2 = pss.tile([128, 512], f32, name="bc2", tag="pssm")
        nc.tensor.matmul(bc2[:], ones_h[:1, 0:128], mgr_h[:1, 450:962],
                         start=True, stop=True)
        bc3 = pss.tile([CAP_S, CAP_E1], f32, name="bc3", tag="pssm")
        nc.tensor.matmul(bc3[:], ones_h[:1, 0:CAP_S], mgr_h[:1, 962:1026],
                         start=True, stop=True)
        ds2 = pss.tile([1, CAP_E0], f32, name="ds2", tag="pssm")
        nc.tensor.matmul(ds2[:], ones_rows[0:3, :1], sqr[:],
                         start=True, stop=True)
        dist = tp.tile([1, CAP_E0], f32, name="dist", tag="dist")
        nc.scalar.sqrt(dist[:], ds2[:])

        # selectors (DVE is_equal against per-partition iota); the combined
        # selAB = [x1R-extract | selA | selB] serves x1's three gathers
        # with a single matmul per chunk.
        sel0s = sb([64, CAP_E0], "sel0s")
        iseq(sel0s[:], bc1[:, 0:384], iota_part[0][:64, :1])
        sel0d = sb([64, CAP_E0], "sel0d")
        iseq(sel0d[:], bc2[0:64, 0:384], iota_part[0][:64, :1])
        selE = []
        for t in range(NT0):
            s_ = sb([128, CAP_E1], f"selE{t}")
            iseq(s_[:], bc2[:, 384:448], iota_part[t][:, :1])
            selE.append(s_)
        selAB = sb([CAP_S, 8 + 2 * CAP_E1], "selAB")
        nc.vector.tensor_copy(selAB[:, 0:8], ident[:CAP_S, 0:R_PER])
        iseq(selAB[:, 8:8 + CAP_E1], bc2[0:CAP_S, 448:512],
             iota_part[0][:CAP_S, :1])
        iseq(selAB[:, 8 + CAP_E1:8 + 2 * CAP_E1], bc3[:, 0:CAP_E1],
             iota_part[0][:CAP_S, :1])

        # ---------------- z normalization -------------------------------
        zsq = tp.tile([64, 512], f32, name="zsq", tag="scr")
        zss = wp.tile([64, 1], f32, name="zss")
        nc.vector.tensor_tensor(zsq[:], mz, mz, op=OP.mult)
        nc.vector.tensor_reduce(zss[:], zsq[:], axis=mybir.AxisListType.X,
                                op=OP.add)
        nc.vector.tensor_scalar(zss[:], zss[:], 1.0 / 512.0, 1e-8,
                                OP.mult, OP.add)
        zsr = wp.tile([64, 1], f32, name="zsr")
        nc.scalar.sqrt(zsr[:], zss[:])
        zrin = wp.tile([64, 1], f32, name="zrin")
        nc.vector.reciprocal(zrin[:], zsr[:])
        zbf = sb([64, 512], "zbf")
        nc.vector.tensor_copy(zbf[:], mz)  # raw z, bf16 (norm later)
        selS = sb([64, CAP_S], "selS")
        iseq(selS[:], bc1[:, 384:450], iota_part[0][:64, :1])
        selSS = sb([64, CAP_S], "selSS")
        nc.vector.tensor_scalar_mul(selSS[:], selS[:], zrin[:, :1])

        # ---------------- PE: z transposes ------------------------------
        # f32 transpose of raw z; the psum copy casts to bf16.
        znT = []
        for k in range(4):
            znT.append(peT(mzc[0:64, 128 * k:128 * (k + 1)], 64, 128,
                           f"znT{k}"))

        # laRhs: feature-major rhs [97 used rows, E0] matching laraw layout
        laRhs = sb([97, CAP_E0], "laRhs")
        nc.gpsimd.memset(laRhs[:], 0.0)
        nc.vector.tensor_copy(laRhs[0:3, :], mgr[0:3, GEO_S:GEO_S + CAP_E0])
        nc.vector.tensor_copy(laRhs[32:35, :], mgr[0:3, GEO_D:GEO_D + CAP_E0])
        nc.vector.tensor_copy(laRhs[64:65, :], dist[:])
        nc.vector.tensor_copy(laRhs[96:97, :],
                              ones_f32[:1, :1].to_broadcast([1, CAP_E0]))

        # token-major la[dst] (for the agg tail) and la[S]
        ldst_bf = sb([128, 9], "ldst_bf")
        nc.scalar.copy(ldst_bf[:], mzc[:, CZ_LDST:CZ_LDST + 9])
        laS_bf = sb([CAP_S, 3], "laS_bf")
        nc.scalar.copy(laS_bf[:], mzc[0:CAP_S, CZ_LAS:CZ_LAS + 3])

        # rhs combo tile for the n00 layer
        rhs_n00 = sb([65, CAP_S], "rhs_n00")
        nc.gpsimd.memset(rhs_n00[:], 0.0)
        nc.vector.tensor_copy(rhs_n00[64:65, :],
                              ones_f32[:1, :1].to_broadcast([1, CAP_S]))
        copyT(laS_bf[:], CAP_S, 3, rhs_n00[0:3, :])

        # ---------------- zterm + zgS (PE; needs GA weights) --------------
        def zterm(base, name):
            ps_zt = psb.tile([64, 512], f32, name=f"ps_{name}", tag="psbig")
            for k in range(4):
                nc.tensor.matmul(ps_zt[:], znT[k][:], W(base + k),
                                 start=(k == 0), stop=(k == 3))
            t_ = sb([64, 512], name)
            # z-norm scale folded into the PSUM->SBUF copy (per-z-row),
            # on the Scalar engine (act copy with per-partition scale)
            nc.scalar.activation(t_[:], ps_zt[:], AF.Copy, bias=0.0,
                                 scale=zrin[:, :1])
            return t_

        ztermA = zterm(T_ZSRC, "ztermA")
        ztermB = zterm(T_ZDST, "ztermB")

        zgS = []
        for c in range(4):
            ps = pss.tile([128, CAP_S], f32, name=f"ps_zg{c}", tag="pssm")
            nc.tensor.matmul(ps[:], zbf[:64, 128 * c:128 * (c + 1)], selSS[:],
                             start=True, stop=True)
            t_ = sb([128, CAP_S], f"zgS{c}")
            ps_copy(t_[:], ps[:])
            zgS.append(t_)

        # ---------------- proc-0 edge MLP layer 1 (feature-major) ---------
        # zterm halves first (need only GA); the laraw half closes each
        # group once GB1 lands, so the PE streams through the GA->GB1 gap.
        h0 = []
        h0ps = []
        for c in range(4):
            cs = slice(128 * c, 128 * (c + 1))
            ps = pss.tile([128, CAP_E0], f32, name=f"ps_efp{c}", tag="pssm")
            nc.tensor.matmul(ps[:], ztermA[:64, cs], sel0s[:],
                             start=True, stop=False)
            nc.tensor.matmul(ps[:], ztermB[:64, cs], sel0d[:],
                             start=False, stop=False)
            h0ps.append(ps)
        for c in range(4):
            cs = slice(128 * c, 128 * (c + 1))
            nc.tensor.matmul(h0ps[c][:], wbig[0:97, T_LARAW, cs],
                             laRhs[0:97, :], start=False, stop=True)
        h0 = [sb([128, CAP_E0], f"h0_{c}") for c in range(4)]

        def h0_chunk(t):
            es = slice(128 * t, 128 * (t + 1))
            for c in range(4):
                tt = tp.tile([128, 128], f32, name=f"lh0_{t}_{c}",
                             tag="lrh0")
                nc.scalar.copy(tt[:], h0ps[c][:, es])
                nc.vector.scalar_tensor_tensor(h0[c][:, es], tt[:], 0.2,
                                               h0ps[c][:, es],
                                               op0=OP.mult, op1=OP.max)

        # ---------------- proc-0 edge MLP layer 2 (token-major) -----------
        # msg tile t needs only e-chunk t of each h0[c]; the h0 chunk
        # lrelus are batched per tile and interleaved with the msg
        # emission, so msg starts ~2us earlier AND each msg tile's own
        # lrelu queues ahead of the next h0 batch -- the psb=2 slot
        # recycling never waits behind unrelated h0 stts on the DVE.
        h0_chunk(0)
        msg = []
        for t in range(NT0):
            es = slice(128 * t, 128 * (t + 1))
            ps_m = psb.tile([128, 512], f32, name=f"ps_ef0{t}", tag="psbig")
            brow_mm(ps_m, "e01", 128)
            for k in range(4):
                nc.tensor.matmul(ps_m[:], h0[k][:, es], W(T_W0E1 + k),
                                 start=False, stop=(k == 3))
            if t + 1 < NT0:
                h0_chunk(t + 1)
            m = sb([128, 512], f"msg{t}")
            lrelu(ps_m[:], m[:], s_copy=True)
            msg.append(m)

        # ---------------- node MLP 0 (FEATURE-major) --------------------
        # out[f, s]: N=66 moving rows per matmul instead of 512, and the
        # fm output chunks are directly the lhsT of the token-major n01 --
        # no transposes.  The zn half opens the accumulations with ready
        # inputs, filling the PE drain while the last msg tile's lrelu is
        # still on the DVE.
        # proc-1 ef0 gathers (need only msg + selE)
        ef0g = []
        for c in range(4):
            ps_g = pss.tile([128, CAP_E1], f32, name=f"ps_ef0g{c}", tag="pssm")
            for t in range(NT0):
                nc.tensor.matmul(ps_g[:], msg[t][:, 128 * c:128 * (c + 1)],
                                 selE[t][:], start=(t == 0),
                                 stop=(t == NT0 - 1))
            o = sb([128, CAP_E1], f"ef0g{c}")
            ps_copy(o[:], ps_g[:])
            ef0g.append(o)

        # ---------------- aggregation onto S (feature-major, rin folded) --
        aggT = []
        for c in range(4):
            ps_ag = pss.tile([128, CAP_S], f32, name=f"ps_agg{c}", tag="pssm")
            for t in range(NT0):
                nc.tensor.matmul(ps_ag[:], msg[t][:, 128 * c:128 * (c + 1)],
                                 G0R[:, CAP_S * t:CAP_S * (t + 1)],
                                 start=(t == 0), stop=(t == NT0 - 1))
            t_ = sb([128, CAP_S], f"aggT{c}")
            ps_copy(t_[:], ps_ag[:])
            aggT.append(t_)
        ps_tl = pss.tile([3, CAP_S], f32, name="ps_aggtl", tag="pssm")
        for t in range(NT0):
            nc.tensor.matmul(ps_tl[:], ldst_bf[:, 3 * t:3 * (t + 1)],
                             G0R[:, CAP_S * t:CAP_S * (t + 1)],
                             start=(t == 0), stop=(t == NT0 - 1))
        ps_copy(rhs_n00[32:35, :], ps_tl[:])

        # ---------------- node MLP 0 (FEATURE-major) --------------------
        # out[f, s]: N=66 moving rows per matmul instead of 512, and the
        # fm output chunks are directly the lhsT of the token-major n01 --
        # no transposes.  Groups run fc-sequentially (one open psum bank
        # at a time); each group's lrelu overlaps the next group's mms.
        hnT = [sb([128, CAP_S], f"hnT{c}") for c in range(4)]
        for fc in range(4):
            fs = slice(128 * fc, 128 * (fc + 1))
            ps_nf = pss.tile([128, CAP_S], f32, name=f"ps_n00f{fc}",
                             tag="pssm")
            for c in range(4):
                nc.tensor.matmul(ps_nf[:], wbig[:, T_W0N0Z + c, fs],
                                 zgS[c][:], start=(c == 0), stop=False)
            for c in range(4):
                nc.tensor.matmul(ps_nf[:], wbig[:, T_W0N0A + c, fs],
                                 aggT[c][:], start=False, stop=False)
            nc.tensor.matmul(ps_nf[:], wbig[0:65, T_N00C, fs],
                             rhs_n00[0:65, :], start=False, stop=True)
            # fm lrelu: stage on Scalar, max on DVE
            t_ = tp.tile([128, CAP_S], f32, name=f"lrn0{fc}", tag="lrnf")
            nc.scalar.copy(t_[:], ps_nf[:])
            nc.vector.scalar_tensor_tensor(hnT[fc][:], t_[:], 0.2,
                                           ps_nf[:],
                                           op0=OP.mult, op1=OP.max)

        # n01; e10 is FEATURE-major (N=64 matmuls), fc-sequential after
        # the x1 gathers; the fm outputs are directly the lhsT of
        # token-major e11, and its per-partition bias folds into the fm
        # lrelu staging on the Scalar engine.
        ps = psb.tile([CAP_S, 512], f32, name="ps_n01", tag="psbig")
        brow_mm(ps, "n01", CAP_S)
        for c in range(4):
            nc.tensor.matmul(ps[:], hnT[c][:], W(T_W0N1 + c),
                             start=False, stop=(c == 3))
        x1tok = sb([CAP_S, 512], "x1tok")
        NG = 8 + 2 * CAP_E1
        x1g = [sb([128, NG], f"x1g{c}") for c in range(4)]
        x1R = [x1g[c][:, 0:R_PER] for c in range(4)]

        # e10-fm fc0/fc1 open early with the ready ef0g half; each x1
        # chunk then feeds their A/B matmuls inline, so the PE streams
        # through the x1-gather round-trips instead of idling.
        h1T = [sb([128, CAP_E1], f"h1T{c}") for c in range(4)]
        be10 = mzc[:, CZ_BE10:CZ_BE10 + 4]
        ps_efE = [pss.tile([128, CAP_E1], f32, name=f"ps_e10f{fc}",
                           tag="pssm") for fc in range(2)]
        for fc in range(2):
            for c in range(4):
                nc.tensor.matmul(ps_efE[fc][:],
                                 wbig8[:, T8_W1E0 + 8 + c,
                                       128 * fc:128 * (fc + 1)],
                                 ef0g[c][:], start=(c == 0), stop=False)

        # 256-col lrelu chunks halve the PE<->DVE<->Scalar round-trips;
        # each chunk feeds two 128-col [x1R | x1(srcE1) | x1(dstE1)]
        # gather matmuls plus fc0's A/B matmuls.
        for c2 in range(2):
            cs = slice(256 * c2, 256 * (c2 + 1))
            t = tp.tile([CAP_S, 256], f32, name=f"lrx1{c2}", tag="lrx1")
            if c2 == 0:
                nc.vector.tensor_copy(t[:], ps[:, cs])
            else:
                nc.scalar.copy(t[:], ps[:, cs])
            nc.vector.scalar_tensor_tensor(x1tok[:, cs], t[:], 0.2,
                                           ps[:, cs],
                                           op0=OP.mult, op1=OP.max)
            for h in range(2):
                c = 2 * c2 + h
                ps_ = pss.tile([128, NG], f32, name=f"ps_x1g{c}",
                               tag="pssm")
                nc.tensor.matmul(ps_[:],
                                 x1tok[:, 128 * c:128 * (c + 1)],
                                 selAB[:], start=True, stop=True)
                nc.vector.tensor_copy(x1g[c][:], ps_[:])
                for fc in range(2):
                    fs0 = slice(128 * fc, 128 * (fc + 1))
                    nc.tensor.matmul(ps_efE[fc][:],
                                     wbig8[:, T8_W1E0 + c, fs0],
                                     x1g[c][:, 8:8 + CAP_E1],
                                     start=False, stop=False)
                    nc.tensor.matmul(ps_efE[fc][:],
                                     wbig8[:, T8_W1E0 + 4 + c, fs0],
                                     x1g[c][:, 8 + CAP_E1:NG],
                                     start=False, stop=(c == 3))
        for fc in range(2):
            t_ = tp.tile([128, CAP_E1], f32, name=f"lre1f{fc}", tag="lref")
            nc.scalar.activation(t_[:], ps_efE[fc][:], AF.Identity,
                                 bias=be10[:, fc:fc + 1], scale=1.0 / F8S)
            nc.vector.scalar_tensor_tensor(h1T[fc][:], t_[:], 0.2, t_[:],
                                           op0=OP.mult, op1=OP.max)
        for fc in range(2, 4):
            fs = slice(128 * fc, 128 * (fc + 1))
            ps_ef = pss.tile([128, CAP_E1], f32, name=f"ps_e10f{fc}",
                             tag="pssm")
            for c in range(4):
                nc.tensor.matmul(ps_ef[:], wbig8[:, T8_W1E0 + c, fs],
                                 x1g[c][:, 8:8 + CAP_E1],
                                 start=(c == 0), stop=False)
                nc.tensor.matmul(ps_ef[:], wbig8[:, T8_W1E0 + 4 + c, fs],
                                 x1g[c][:, 8 + CAP_E1:NG],
                                 start=False, stop=False)
            for c in range(4):
                nc.tensor.matmul(ps_ef[:], wbig8[:, T8_W1E0 + 8 + c, fs],
                                 ef0g[c][:], start=False, stop=(c == 3))
            # fm lrelu with per-partition bias: t = psum/F8S + b, max on DVE
            t_ = tp.tile([128, CAP_E1], f32, name=f"lre1{fc}", tag="lref")
            nc.scalar.activation(t_[:], ps_ef[:], AF.Identity,
                                 bias=be10[:, fc:fc + 1], scale=1.0 / F8S)
            nc.vector.scalar_tensor_tensor(h1T[fc][:], t_[:], 0.2, t_[:],
                                           op0=OP.mult, op1=OP.max)

        # e11 (token-major; lhsT = fm h1 chunks directly); each chunk
        # feeds the R-aggregation feature-major (rin1 folded into G1R) and
        # then its 4 fm n10 matmuls -- no transposes anywhere.
        msg1 = sb([CAP_E1, 512], "msg1")
        ps_e11 = psb.tile([CAP_E1, 512], f32, name="ps_e11", tag="psbig")
        brow_mm(ps_e11, "e11", CAP_E1)
        for c in range(4):
            nc.tensor.matmul(ps_e11[:], h1T[c][:], W(T_W1E1 + c),
                             start=False, stop=(c == 3))
        agg1T = [sb([128, R_PER], f"agg1T{c}") for c in range(4)]
        # n10-fm fc0/fc1 open early with the ready x1R half; each agg1T
        # then feeds their matmuls inline, filling e11's lrelu round-trips.
        bn10 = mzc[:, CZ_BN10:CZ_BN10 + 4]
        ps_ntE = [pss.tile([128, R_PER], f32, name=f"ps_n10f{fc}",
                           tag="pssm") for fc in range(2)]
        for fc in range(2):
            for c in range(4):
                nc.tensor.matmul(ps_ntE[fc][:],
                                 wbig[:, T_W1N0 + c,
                                      128 * fc:128 * (fc + 1)],
                                 x1g[c][:, 0:R_PER],
                                 start=(c == 0), stop=False)
        for c2 in range(2):
            cs = slice(256 * c2, 256 * (c2 + 1))
            t = tp.tile([CAP_E1, 256], f32, name=f"lre11{c2}", tag="lre2")
            if c2 == 0:
                nc.vector.tensor_copy(t[:], ps_e11[:, cs])
            else:
                nc.scalar.copy(t[:], ps_e11[:, cs])
            nc.vector.scalar_tensor_tensor(msg1[:, cs], t[:], 0.2,
                                           ps_e11[:, cs],
                                           op0=OP.mult, op1=OP.max)
            for h in range(2):
                c = 2 * c2 + h
                ps_ = pss.tile([128, R_PER], f32, name=f"ps_ag1{c}",
                               tag="pssm")
                nc.tensor.matmul(ps_[:],
                                 msg1[:, 128 * c:128 * (c + 1)],
                                 G1R, start=True, stop=True)
                nc.vector.tensor_copy(agg1T[c][:], ps_[:])
                for fc in range(2):
                    nc.tensor.matmul(ps_ntE[fc][:],
                                     wbig[:, T_W1N0 + 4 + c,
                                          128 * fc:128 * (fc + 1)],
                                     agg1T[c][:], start=False,
                                     stop=(c == 3))

        # ---------------- final node MLPs (fm n10 -> token n11) -----------
        hfT = [sb([128, R_PER], f"hfT{c}") for c in range(4)]
        ps_n11 = psb.tile([R_PER, 512], f32, name="ps_n11", tag="psbig")
        brow_mm(ps_n11, "n11", R_PER)
        for fc in range(2):
            t_ = tp.tile([128, R_PER], f32, name=f"lrnf{fc}", tag="lrn1")
            nc.scalar.activation(t_[:], ps_ntE[fc][:], AF.Identity,
                                 bias=bn10[:, fc:fc + 1], scale=1.0)
            nc.vector.scalar_tensor_tensor(hfT[fc][:], t_[:], 0.2, t_[:],
                                           op0=OP.mult, op1=OP.max)
            nc.tensor.matmul(ps_n11[:], hfT[fc][:], W(T_W1N1 + fc),
                             start=False, stop=False)
        for fc in range(2, 4):
            fs = slice(128 * fc, 128 * (fc + 1))
            ps_nt = pss.tile([128, R_PER], f32, name=f"ps_n10f{fc}",
                             tag="pssm")
            for c in range(4):
                nc.tensor.matmul(ps_nt[:], wbig[:, T_W1N0 + c, fs],
                                 x1g[c][:, 0:R_PER],
                                 start=(c == 0), stop=False)
            for c in range(4):
                nc.tensor.matmul(ps_nt[:], wbig[:, T_W1N0 + 4 + c, fs],
                                 agg1T[c][:], start=False, stop=(c == 3))
            t_ = tp.tile([128, R_PER], f32, name=f"lrnf{fc}", tag="lrn1")
            nc.scalar.activation(t_[:], ps_nt[:], AF.Identity,
                                 bias=bn10[:, fc:fc + 1], scale=1.0)
            nc.vector.scalar_tensor_tensor(hfT[fc][:], t_[:], 0.2, t_[:],
                                           op0=OP.mult, op1=OP.max)
            nc.tensor.matmul(ps_n11[:], hfT[fc][:], W(T_W1N1 + fc),
                             start=False, stop=(fc == 3))
        wstok = wp.tile([R_PER, 512], f32, name="wstok")
        lrelu(ps_n11[:], wstok[:])

        nc.sync.dma_start(out_d[:, :], wstok[:, :])

        if DEBUG_DUMPS:
            for nm, t_ in [("ztermA", ztermA), ("ztermB", ztermB),
                           ("h0_0", h0[0]), ("msg0", msg[0]),
                           ("aggT0", aggT[0]), ("hnT0", hnT[0]),
                           ("x1tok", x1tok), ("h1T0", h1T[0]),
                           ("msg1", msg1), ("hfT0", hfT[0]),
                           ("laRhs", laRhs),
                           ("zgS0", zgS[0]), ("rhs_n00", rhs_n00),
                           ("sel0s", sel0s), ("agg1T0", agg1T[0]),
                           ("ef0g0", ef0g[0]), ("x1g0", x1g[0])]:
                shp = list(t_.shape)
                dd = nc.dram_tensor(f"dbg_{nm}", shp, t_.dtype,
                                    kind="ExternalOutput")
                nc.sync.dma_start(dd[:, :], t_[:, :])

    nc.finalize()
    return nc


_PROG_CACHE = {}


def _get_program():
    key = (CAP_E0, CAP_S, CAP_E1)
    if key not in _PROG_CACHE:
        _PROG_CACHE[key] = _build_program()
    return _PROG_CACHE[key]


def _pad(a, n, fill):
    out = np.full((n,), fill, dtype=np.float32)
    out[:len(a)] = a.astype(np.float32)
    return out


def _host_weights(inputs):
    """Pack all FC weights (transposed, gain*sqrt2 pre-folded) + biases
    into one [NT*128, 512] bf16 tensor of K-tiles."""
    f = np.float32
    s = SQ2

    def T(name):
        return np.ascontiguousarray(np.asarray(inputs[name], f).T)

    w0e0T, w0e1T = T("p0_ew0"), T("p0_ew1")
    w0n0T, w0n1T = T("p0_nw0"), T("p0_nw1")
    w1e0T, w1e1T = T("p1_ew0"), T("p1_ew1")
    w1n0T, w1n1T = T("p1_nw0"), T("p1_nw1")

    def bias(name):
        return np.asarray(inputs[name], f)

    wpk = np.zeros((NT * 128, 512), f)

    def put(idx, rows):
        wpk[idx * 128: idx * 128 + rows.shape[0]] = rows

    put(T_ZSRC, w0e0T[0:512] * (G_E00 * s))
    put(T_ZDST, w0e0T[515:1027] * (G_E00 * s))
    for key, bname in [("e01", "p0_eb1"), ("n01", "p0_nb1"),
                       ("e10", "p1_eb0"), ("e11", "p1_eb1"),
                       ("n10", "p1_nb0"), ("n11", "p1_nb1")]:
        tidx, pbase = BROW_SLOT[key]
        bsc = F8S if key == "e10" else 1.0
        wpk[tidx * 128 + pbase] = bias(bname) * (LR * s * bsc)
    # rel = la[dst]-la[src] folds into the src/dst la blocks:
    #   src rows get (laA - w_rel), dst rows get (laB + w_rel)
    laraw = np.zeros((128, 512), f)
    laraw[0:3] = (w0e0T[512:515] - w0e0T[1030:1033]) * (G_E00 * s)
    laraw[32:35] = (w0e0T[1027:1030] + w0e0T[1030:1033]) * (G_E00 * s)
    laraw[64:65] = w0e0T[1033:1034] * (G_E00 * s)  # dist weight
    laraw[96] = bias("p0_eb0") * (LR * s)
    put(T_LARAW, laraw)
    put(T_W0E1, w0e1T * (G_E01 * s))
    put(T_W0N0Z, w0n0T[0:512] * (G_N00 * s))
    # n00 input dims: 0:512 zn | 512:515 la | 515:518 la_dst-mean | 518:1030
    # ef-mean.  aggT holds the ef-mean block, rhs_n00[32:35] the la_dst-mean.
    put(T_W0N0A, w0n0T[518:1030] * (G_N00 * s))
    comb = np.zeros((128, 512), f)
    comb[0:3] = w0n0T[512:515] * (G_N00 * s)    # la features of x
    comb[32:35] = w0n0T[515:518] * (G_N00 * s)  # la_dst-mean
    comb[64] = bias("p0_nb0") * (LR * s)
    put(T_N00C, comb)
    put(T_W0N1, w0n1T * (G_N01 * s))
    put(T_W1E1, w1e1T * (G_E11 * s))
    put(T_W1N0, w1n0T * (G_N10 * s))
    put(T_W1N1, w1n1T * (G_N11 * s))
    wpk8 = np.zeros((NT8 * 128, 512), f)
    wpk8[T8_W1E0 * 128:(T8_W1E0 + 12) * 128] = w1e0T * (G_E10 * s * F8S)
    wpk8 = wpk8.reshape(NT8 // 4, 4, 128, 512).transpose(0, 2, 1, 3)
    wpk8 = np.ascontiguousarray(wpk8.reshape(NT8 * 128, 512))
    wpk8 = np.ascontiguousarray(wpk8.astype(ml_dtypes.float8_e4m3))
    # pair-interleave rows: tile pair q -> rows (q*128+p)*2+j
    wpk = wpk.reshape(NT // 2, 2, 128, 512).transpose(0, 2, 1, 3)
    wpk = np.ascontiguousarray(wpk.reshape(NT * 128, 512))
    return np.ascontiguousarray(wpk.astype(ml_dtypes.bfloat16)), wpk8


def _core_meta(z, la, src, dst, c, bias_fm):
    """Per-core metadata tensors (integer index-set construction + row
    gathers of input data + 1/count fold; no arithmetic on tensor values)."""
    Rc = (np.arange(R_PER, dtype=np.int64) + c * R_PER) * NV
    E1 = np.nonzero(np.isin(dst, Rc))[0]
    others = np.setdiff1d(np.unique(src[E1]), Rc)
    S = np.concatenate([Rc, others])
    assert len(E1) <= CAP_E1 and len(S) <= CAP_S, (len(E1), len(S))
    slot = np.full(16000, -1, np.int64)
    slot[S] = np.arange(len(S))
    E0 = np.nonzero(slot[dst] >= 0)[0]
    assert len(E0) <= CAP_E0, len(E0)
    pos = np.full(src.shape[0], -1, np.int64)
    pos[E0] = np.arange(len(E0))
    e0s, e0d = src[E0], dst[E0]
    e1s, e1d = src[E1], dst[E1]

    def gat(idx, n):
        out = np.zeros((n, 3), np.float32)
        out[:len(idx)] = la[idx]
        return out

    # rin-folded one-hot gather matrices (bf16, bit-packed into f32 cols)
    cnt0 = np.bincount(slot[e0d].astype(np.int64), minlength=CAP_S)[:CAP_S]
    rin0 = (1.0 / np.maximum(cnt0, 1)).astype(np.float32)
    sig0 = _pad(slot[e0d], CAP_E0, -1).astype(np.int64)
    G0R = np.zeros((128, NT0 * CAP_S), np.float32)
    for t in range(NT0):
        blk = sig0[128 * t:128 * (t + 1)]
        for e in range(128):
            if blk[e] >= 0:
                G0R[e, CAP_S * t + blk[e]] = rin0[blk[e]]
    cnt1 = np.bincount(slot[e1d].astype(np.int64), minlength=R_PER)[:R_PER]
    rin1 = (1.0 / np.maximum(cnt1, 1)).astype(np.float32)
    G1R = np.zeros((CAP_E1, R_PER), np.float32)
    for e in range(len(E1)):
        G1R[e, slot[e1d[e]]] = rin1[slot[e1d[e]]]

    def pack_bf16(a, rows):
        b = np.zeros((rows, a.shape[1]), ml_dtypes.bfloat16)
        b[:a.shape[0]] = a.astype(ml_dtypes.bfloat16)
        if b.shape[1] % 2:
            b = np.concatenate(
                [b, np.zeros((rows, 1), ml_dtypes.bfloat16)], axis=1)
        return np.ascontiguousarray(b).view(np.float32)

    mzc = np.zeros((128, MZC_F), np.float32)
    mzc[0:64, 0:512] = z
    la_d = gat(e0d, CAP_E0).reshape(NT0, 128, 3)
    for t in range(NT0):
        mzc[:, CZ_LDST + 3 * t:CZ_LDST + 3 * (t + 1)] = la_d[t]
    mzc[0:CAP_S, CZ_LAS:CZ_LAS + 3] = gat(S, CAP_S)
    mzc[0:CAP_E1, CZ_G1R:CZ_G1R + 4] = pack_bf16(G1R, CAP_E1)
    mzc[:, CZ_G0R:CZ_G0R + 99] = pack_bf16(G0R, 128)
    mzc[:, CZ_BE10:CZ_BE10 + 12] = bias_fm

    mgr = np.zeros((3, MGR_F), np.float32)
    mgr[0:3, GEO_S:GEO_S + CAP_E0] = gat(e0s, CAP_E0).T
    mgr[0:3, GEO_D:GEO_D + CAP_E0] = gat(e0d, CAP_E0).T
    mrow = np.zeros(MR_N, np.float32)
    mrow[MR_E0GS:MR_E0GS + CAP_E0] = _pad(e0s % B, CAP_E0, -1)
    mrow[MR_SSEL:MR_SSEL + CAP_S] = _pad(S % B, CAP_S, -1)
    mrow[MR_E0GD:MR_E0GD + CAP_E0] = _pad(e0d % B, CAP_E0, -1)
    mrow[MR_E1POS:MR_E1POS + CAP_E1] = _pad(pos[E1], CAP_E1, -1)
    mrow[MR_E1SRC:MR_E1SRC + CAP_E1] = _pad(slot[e1s], CAP_E1, -1)
    mrow[MR_E1DST:MR_E1DST + CAP_E1] = _pad(slot[e1d], CAP_E1, -1)
    mgr[0, MR0:MR0 + MR_N] = mrow

    return {"mzc": mzc,
            "mgr": np.ascontiguousarray(mgr.reshape(24, MGR_F // 8))}


def make_in_maps(inputs):
    ei = np.asarray(inputs["edge_index"])
    src, dst = ei[0].astype(np.int64), ei[1].astype(np.int64)
    z = np.ascontiguousarray(np.asarray(inputs["z"], np.float32))
    la = np.ascontiguousarray(np.asarray(inputs["look_ats"], np.float32))
    wpk, wpk8 = _host_weights(inputs)
    s = np.sqrt(2.0, dtype=np.float32)
    bias_fm = np.stack([
        np.asarray(inputs["p1_eb0"], np.float32).reshape(4, 128).T,
        np.asarray(inputs["p1_nb0"], np.float32).reshape(4, 128).T,
        np.asarray(inputs["p1_nb1"], np.float32).reshape(4, 128).T,
    ], axis=1).reshape(128, 12) * (LR * s)
    return [dict(wpack=wpk, wpack8=wpk8,
                 **_core_meta(z, la, src, dst, c, bias_fm))
            for c in range(N_CORES)]


def kernel(**inputs):
    nc = _get_program()
    in_maps = make_in_maps(inputs)
    res = run_bass_kernel_spmd(nc, in_maps, core_ids=list(range(N_CORES)))
    ws = np.concatenate([res.results[c]["out"] for c in range(N_CORES)],
                        axis=0).astype(np.float32)
    return np.ascontiguousarray(
        np.broadcast_to(ws[:, None, :], (B, 14, D))).astype(np.float32)
